# revision 16
# baseline (speedup 1.0000x reference)
# Channel-Attention Module (CAM) kernel for Trainium2, 8 NeuronCores.
#
# reference:
#   a   = x.reshape(B, N, C)                 # B=16, N=64*64=4096, C=512
#   G   = einsum('bnc,bnd->bcd', a, a)       # [B, C, C]
#   att = softmax(G, axis=-1)
#   out = gamma * einsum('bnc,bcd->bnd', a, att) + x
#
# Sharding: data-parallel over batch, 2 samples per core.
#
# Numerics: a @ att is rewritten as a @ R + a with R = att - I.  The
# dominant identity contribution is applied as an f32 elementwise op, so
# only a @ R runs through the bf16 tensor engine.  Folding gamma in:
#   out = a @ (gamma*(att - I)) + (1 + gamma) * a
# Both big matmuls (G and a@R) run in bf16 on the PE at full rate; the
# softmax and all elementwise work is f32.
#
# G is symmetric (exactly, even with bf16 inputs), so only the upper
# block-triangle is computed on the PE (d >= 128*m for row-block m); the
# lower blocks are filled in by PE-transposing the mirror blocks while
# assembling full softmax rows in SBUF.
#
# Engine placement: input DMAs ride the ACT HWDGE ring, output DMAs the
# SP ring (independent FIFOs).  Casts/exp on ACT, reductions/adds on
# DVE, both big matmuls plus the a^T transpose (regular matmul against
# a bf16 identity) on the PE.

from contextlib import ExitStack

import numpy as np
import ml_dtypes

B = 16
HW_H = 64
HW_W = 64
N = HW_H * HW_W          # 4096 pixels per sample
C = 512                  # channels
NCORES = 8
SPC = B // NCORES        # samples per core
P = 128                  # partitions
MT = C // P              # 4 c-tiles
NT = N // P              # 32 n-tiles per sample
NG = NT // 4             # 8 groups of 4 n-tiles (1MB DMA granularity)

_CACHE = {}


def _build(repeat=1):
    # repeat>1 re-runs the whole shard computation inside one NEFF; used
    # only by the timing harness (work-delta cancels dispatch overhead).
    import concourse.bacc as bacc
    import concourse.tile as tile
    import concourse.mybir as mybir

    fp32 = mybir.dt.float32
    bf16 = mybir.dt.bfloat16
    AX = mybir.AxisListType.X
    OP = mybir.AluOpType
    AF = mybir.ActivationFunctionType

    nc = bacc.Bacc(
        "TRN2",
        target_bir_lowering=False,
        debug=False,
        enable_asserts=False,
        num_devices=NCORES,
    )
    x_d = nc.dram_tensor("x", [SPC * N, C], fp32, kind="ExternalInput").ap()
    gvec_d = nc.dram_tensor("gvec", [P, 1], fp32, kind="ExternalInput").ap()
    gp1_d = nc.dram_tensor("gp1vec", [P, 1], fp32, kind="ExternalInput").ap()
    ident_d = nc.dram_tensor("ident", [P, P], bf16, kind="ExternalInput").ap()
    identf_d = nc.dram_tensor("identf", [P, P], fp32, kind="ExternalInput").ap()
    gi_d = nc.dram_tensor("gI", [P, P], fp32, kind="ExternalInput").ap()
    out_d = nc.dram_tensor("out", [SPC * N, C], fp32, kind="ExternalOutput").ap()

    with tile.TileContext(nc) as tc, ExitStack() as ctx:
        p_const = ctx.enter_context(tc.tile_pool(name="pconst", bufs=1))
        p_a = ctx.enter_context(tc.tile_pool(name="pa", bufs=1))
        p_a2 = ctx.enter_context(tc.tile_pool(name="pa2", bufs=2))
        p_a16 = ctx.enter_context(tc.tile_pool(name="pa16", bufs=1))
        p_aT = ctx.enter_context(tc.tile_pool(name="paT", bufs=1))
        p_gs = ctx.enter_context(tc.tile_pool(name="pgs", bufs=1))
        p_sm = ctx.enter_context(tc.tile_pool(name="psm", bufs=2))
        p_R = ctx.enter_context(tc.tile_pool(name="pR", bufs=1))
        p_st = ctx.enter_context(tc.tile_pool(name="pst", bufs=4))
        pp_g = ctx.enter_context(tc.tile_pool(name="ppg", bufs=2, space="PSUM"))
        pp_t = ctx.enter_context(tc.tile_pool(name="ppt", bufs=2, space="PSUM"))
        pp_o = ctx.enter_context(tc.tile_pool(name="ppo", bufs=2, space="PSUM"))

        gvec = p_const.tile([P, 1], fp32, name="gvec_sb")
        nc.sync.dma_start(out=gvec, in_=gvec_d)
        gp1 = p_const.tile([P, 1], fp32, name="gp1_sb")
        nc.sync.dma_start(out=gp1, in_=gp1_d)
        ident = p_const.tile([P, P], bf16, name="ident_sb")
        nc.sync.dma_start(out=ident, in_=ident_d)
        identf = p_const.tile([P, P], fp32, name="identf_sb")
        nc.sync.dma_start(out=identf, in_=identf_d)
        gI = p_const.tile([P, P], fp32, name="gI_sb")
        nc.sync.dma_start(out=gI, in_=gi_d)

        # aT is allocated once and reused across samples: per-sample
        # re-allocation would add a whole-tile release dependency (next
        # sample's transposes waiting on ALL of the previous out2 reads),
        # while a persistent tile gets byte-range WAR tracking, which the
        # D/A interleave below satisfies just-in-time.
        aT_tiles = [
            p_aT.tile([P, N], bf16, tag=f"aT_{j}", name=f"aT_{j}")
            for j in range(MT)
        ]

        # Per-sample state: af (resident f32 a / z buffers), a16, aT, Rb.
        state = {}

        def emit_A_group(sl, g):
            # load 1MB group g (ACT HWDGE ring) into the resident f32
            # buffer, cast to bf16, PE-transpose its 4x4 blocks into aT,
            # and feed its k-tiles into G row-block m=0.
            st = state[sl]
            s = sl % SPC
            pool = p_a2 if g < 2 else p_a
            xt = pool.tile([P, 4, C], fp32, tag=f"a_{g}", name=f"a_{sl}_{g}")
            src = x_d[s * N + g * 512 : s * N + (g + 1) * 512, :].rearrange(
                "(u p) c -> p u c", p=P
            )
            a = p_a16.tile([P, 4, C], bf16, tag=f"a16_{g}", name=f"a16_{sl}_{g}")
            if sl == 0 and g == 0:
                # first group: 256KB pieces so the PE starts ~3us earlier
                for u in range(4):
                    nc.scalar.dma_start(out=xt[:, u, :], in_=src[:, u, :])
                    nc.scalar.activation(a[:, u, :], xt[:, u, :], AF.Copy)
            else:
                nc.scalar.dma_start(out=xt, in_=src)
                nc.scalar.activation(a, xt, AF.Copy)
            st["af"].append(xt)
            st["a16"].append(a)
            for u in range(4):
                k = 4 * g + u
                nc.tensor.matmul(
                    st["psg0"],
                    a[:, u, 0:P],
                    a[:, u, :],
                    start=(k == 0),
                    stop=(k == NT - 1),
                )
            for j in range(MT):
                pst = pp_t.tile([P, C], fp32, tag="pst", name=f"pst_{sl}_{j}_{g}")
                for u in range(4):
                    nc.tensor.matmul(
                        pst[:, u * P : (u + 1) * P],
                        a[:, u, j * P : (j + 1) * P],
                        ident,
                        start=(u == 0),
                        stop=(u == 3),
                    )
                dstc = st["aT"][j][:, g * 512 : (g + 1) * 512]
                if j == MT - 1:
                    nc.scalar.activation(dstc, pst, AF.Copy)
                else:
                    nc.vector.tensor_copy(dstc, pst)

        def emit_B(sl):
            # remaining G row-blocks (upper triangle only; G is exactly
            # symmetric), mirror lower blocks via PE transpose, softmax
            # rows, R = gamma*(att - I) in bf16.
            st = state[sl]
            a16 = st["a16"]
            Gs = []
            for m in range(MT):
                w = C - m * P
                if m == 0:
                    psg = st["psg0"]
                else:
                    psg = pp_g.tile([P, w], fp32, tag="psg", name=f"psg_{sl}_{m}")
                    for k in range(NT):
                        g, u = divmod(k, 4)
                        nc.tensor.matmul(
                            psg,
                            a16[g][:, u, m * P : (m + 1) * P],
                            a16[g][:, u, m * P :],
                            start=(k == 0),
                            stop=(k == NT - 1),
                        )
                gs = p_gs.tile([P, C], fp32, tag=f"Gs_{m}", name=f"Gs_{sl}_{m}")
                nc.scalar.activation(gs[:, m * P :], psg, AF.Copy)
                for j in range(m):
                    tps = pp_t.tile([P, P], fp32, tag="pst", name=f"tps_{sl}_{m}_{j}")
                    nc.tensor.transpose(tps, Gs[j][:, m * P : (m + 1) * P], identf)
                    nc.scalar.activation(gs[:, j * P : (j + 1) * P], tps, AF.Copy)
                Gs.append(gs)

                nmax = p_st.tile([P, 1], fp32, tag="nmax", name=f"nmax_{sl}_{m}")
                nc.vector.reduce_max(nmax, gs, axis=AX, negate=True)
                E = p_sm.tile([P, C], fp32, tag="E", name=f"E_{sl}_{m}")
                nc.scalar.activation(E, gs, AF.Exp, bias=nmax, scale=1.0)
                ssum = p_st.tile([P, 1], fp32, tag="ssum", name=f"ssum_{sl}_{m}")
                nc.vector.reduce_sum(ssum, E, axis=AX)
                rin = p_st.tile([P, 1], fp32, tag="rin", name=f"rin_{sl}_{m}")
                nc.vector.reciprocal(rin, ssum)
                ag = p_sm.tile([P, C], fp32, tag="ag", name=f"ag_{sl}_{m}")
                nc.vector.tensor_scalar(ag, E, rin, gvec, OP.mult, OP.mult)
                nc.vector.tensor_sub(
                    ag[:, m * P : (m + 1) * P], ag[:, m * P : (m + 1) * P], gI
                )
                r = p_R.tile([P, C], bf16, tag=f"R_{m}", name=f"R_{sl}_{m}")
                nc.scalar.activation(r, ag, AF.Copy)
                st["Rb"].append(r)

        def emit_D_pair(sl, v):
            # tmp = a @ R for n-tiles (2v, 2v+1); z = (1+gamma)*a + tmp
            # fused on DVE, in place over the f32 a buffer; store per 1MB.
            st = state[sl]
            s = sl % SPC
            po = pp_o.tile([P, 2, C], fp32, tag="po", name=f"po_{sl}_{v}")
            for t in range(2):
                i = 2 * v + t
                for k in range(MT):
                    nc.tensor.matmul(
                        po[:, t, :],
                        st["aT"][k][:, i * P : (i + 1) * P],
                        st["Rb"][k],
                        start=(k == 0),
                        stop=(k == MT - 1),
                    )
            gg, vv = divmod(2 * v, 4)
            zsl = st["af"][gg][:, vv : vv + 2, :]
            nc.vector.scalar_tensor_tensor(zsl, zsl, gp1, po, OP.mult, OP.add)
            if vv == 2:
                dst = out_d[
                    s * N + gg * 512 : s * N + (gg + 1) * 512, :
                ].rearrange("(u p) c -> p u c", p=P)
                nc.sync.dma_start(out=dst, in_=st["af"][gg])

        total = SPC * repeat
        for sl in range(total):
            state[sl] = {
                "af": [],
                "a16": [],
                "Rb": [],
                "aT": aT_tiles,
                "psg0": pp_g.tile([P, C], fp32, tag="psg", name=f"psg_{sl}_0"),
            }
            # Interleave the previous sample's phase D with this sample's
            # load/transpose loop: out2 pairs (2g, 2g+1) are the last
            # readers of exactly the aT columns group g overwrites, so
            # emitting them just ahead keeps every dependency just-in-time.
            for g in range(NG):
                if sl > 0:
                    emit_D_pair(sl - 1, 2 * g)
                    emit_D_pair(sl - 1, 2 * g + 1)
                emit_A_group(sl, g)
            if sl > 0:
                state.pop(sl - 1)
            emit_B(sl)
        for v in range(NT // 2):
            emit_D_pair(total - 1, v)

    nc.compile()
    return nc


def _get_nc():
    if "nc" not in _CACHE:
        _CACHE["nc"] = _build()
    return _CACHE["nc"]


def _in_maps(x, gamma):
    x = np.asarray(x).astype(np.float32, copy=False)
    g = np.float32(np.asarray(gamma).reshape(-1)[0])
    xs = x.reshape(B, N, C)
    gvec = np.full((P, 1), g, np.float32)
    gp1 = np.full((P, 1), np.float32(1.0) + g, np.float32)
    ident = np.eye(P, dtype=ml_dtypes.bfloat16)
    identf = np.eye(P, dtype=np.float32)
    gi = (g * np.eye(P)).astype(np.float32)
    maps = []
    for r in range(NCORES):
        shard = np.ascontiguousarray(xs[r * SPC : (r + 1) * SPC].reshape(SPC * N, C))
        maps.append(
            {
                "x": shard,
                "gvec": gvec,
                "gp1vec": gp1,
                "ident": ident,
                "identf": identf,
                "gI": gi,
            }
        )
    return maps


def _run(x, gamma, trace=False):
    import os

    if not trace:
        # the NTFF trace hook (antenv.axon_hooks) is absent in this axon
        # build; make sure an inherited BASS_TRACE can't route us there
        os.environ.setdefault("BASS_NEVER_TRACE", "1")
    from concourse import bass_utils

    nc = _get_nc()
    res = bass_utils.run_bass_kernel_spmd(
        nc, _in_maps(x, gamma), core_ids=list(range(NCORES)), trace=trace
    )
    out = np.concatenate(
        [res.results[r]["out"].reshape(SPC, N, C) for r in range(NCORES)], axis=0
    )
    return out.reshape(B, HW_H, HW_W, C).astype(np.float32, copy=False), res


def kernel(x, gamma):
    out, _ = _run(x, gamma, trace=False)
    return out


# revision 26
# speedup vs baseline: 1.1832x; 1.1832x over previous
# Channel-Attention Module (CAM) kernel for Trainium2, 8 NeuronCores.
#
# reference:
#   a   = x.reshape(B, N, C)                 # B=16, N=64*64=4096, C=512
#   G   = einsum('bnc,bnd->bcd', a, a)       # [B, C, C]
#   att = softmax(G, axis=-1)
#   out = gamma * einsum('bnc,bcd->bnd', a, att) + x
#
# Sharding: data-parallel over batch, 2 samples per core.
#
# Numerics: a @ att is rewritten as a @ R + a with R = att - I.  The
# dominant identity contribution is applied as an f32 elementwise op, so
# only a @ R runs through the bf16 tensor engine.  Folding gamma in:
#   out = a @ (gamma*(att - I)) + (1 + gamma) * a
# Both big matmuls (G and a@R) run in bf16 on the PE at full rate; the
# softmax and all elementwise work is f32.
#
# G is symmetric (exactly, even with bf16 inputs), so only the upper
# block-triangle is computed on the PE (d >= 128*m for row-block m); the
# lower blocks are filled in by PE-transposing the mirror blocks while
# assembling full softmax rows in SBUF.
#
# Engine placement: input/output DMAs ride the SP HWDGE ring (a
# dispatch-only stream — queuing them on ACT delays loads behind ACT
# compute), constants go via SWDGE.  Casts/exp and half the transpose
# copybacks on ACT, reductions/adds and the other copybacks on DVE,
# both big matmuls plus the a^T transpose (regular matmul against a
# bf16 identity) on the PE.  All four G row-blocks accumulate in four
# concurrently-open PSUM banks while the load loop streams, so the PE
# never starves waiting for the next 1MB group.

from contextlib import ExitStack

import numpy as np
import ml_dtypes

B = 16
HW_H = 64
HW_W = 64
N = HW_H * HW_W          # 4096 pixels per sample
C = 512                  # channels
NCORES = 8
SPC = B // NCORES        # samples per core
P = 128                  # partitions
MT = C // P              # 4 c-tiles
NT = N // P              # 32 n-tiles per sample
NG = NT // 4             # 8 groups of 4 n-tiles (1MB DMA granularity)

_CACHE = {}


def _build(repeat=1):
    # repeat>1 re-runs the whole shard computation inside one NEFF; used
    # only by the timing harness (work-delta cancels dispatch overhead).
    import concourse.bacc as bacc
    import concourse.tile as tile
    import concourse.mybir as mybir

    fp32 = mybir.dt.float32
    bf16 = mybir.dt.bfloat16
    AX = mybir.AxisListType.X
    OP = mybir.AluOpType
    AF = mybir.ActivationFunctionType

    nc = bacc.Bacc(
        "TRN2",
        target_bir_lowering=False,
        debug=False,
        enable_asserts=False,
        num_devices=NCORES,
    )
    x_d = nc.dram_tensor("x", [SPC * N, C], fp32, kind="ExternalInput").ap()
    gvec_d = nc.dram_tensor("gvec", [P, 1], fp32, kind="ExternalInput").ap()
    gp1_d = nc.dram_tensor("gp1vec", [P, 1], fp32, kind="ExternalInput").ap()
    ident_d = nc.dram_tensor("ident", [P, P], bf16, kind="ExternalInput").ap()
    identf_d = nc.dram_tensor("identf", [P, P], fp32, kind="ExternalInput").ap()
    gi_d = nc.dram_tensor("gI", [P, P], fp32, kind="ExternalInput").ap()
    out_d = nc.dram_tensor("out", [SPC * N, C], fp32, kind="ExternalOutput").ap()

    with tile.TileContext(nc) as tc, ExitStack() as ctx:
        p_const = ctx.enter_context(tc.tile_pool(name="pconst", bufs=1))
        p_a = ctx.enter_context(tc.tile_pool(name="pa", bufs=1))
        p_a2 = ctx.enter_context(tc.tile_pool(name="pa2", bufs=2))
        p_a16 = ctx.enter_context(tc.tile_pool(name="pa16", bufs=1))
        p_aT = ctx.enter_context(tc.tile_pool(name="paT", bufs=1))
        p_gs = ctx.enter_context(tc.tile_pool(name="pgs", bufs=1))
        p_sm = ctx.enter_context(tc.tile_pool(name="psm", bufs=2))
        p_R = ctx.enter_context(tc.tile_pool(name="pR", bufs=1))
        p_st = ctx.enter_context(tc.tile_pool(name="pst", bufs=4))
        pp_g = ctx.enter_context(tc.tile_pool(name="ppg", bufs=4, space="PSUM"))
        pp_t = ctx.enter_context(tc.tile_pool(name="ppt", bufs=2, space="PSUM"))
        pp_o = ctx.enter_context(tc.tile_pool(name="ppo", bufs=2, space="PSUM"))

        gvec = p_const.tile([P, 1], fp32, name="gvec_sb")
        nc.gpsimd.dma_start(out=gvec, in_=gvec_d)
        gp1 = p_const.tile([P, 1], fp32, name="gp1_sb")
        nc.gpsimd.dma_start(out=gp1, in_=gp1_d)
        ident = p_const.tile([P, P], bf16, name="ident_sb")
        nc.gpsimd.dma_start(out=ident, in_=ident_d)
        identf = p_const.tile([P, P], fp32, name="identf_sb")
        nc.gpsimd.dma_start(out=identf, in_=identf_d)
        gI = p_const.tile([P, P], fp32, name="gI_sb")
        nc.gpsimd.dma_start(out=gI, in_=gi_d)

        # aT is allocated once and reused across samples: per-sample
        # re-allocation would add a whole-tile release dependency (next
        # sample's transposes waiting on ALL of the previous out2 reads),
        # while a persistent tile gets byte-range WAR tracking, which the
        # D/A interleave below satisfies just-in-time.
        aT_tiles = [
            p_aT.tile([P, N], bf16, tag=f"aT_{j}", name=f"aT_{j}")
            for j in range(MT)
        ]

        # Per-sample state: af (resident f32 a / z buffers), a16, aT, Rb.
        state = {}

        def emit_A_group(sl, g):
            # load 1MB group g (SP HWDGE ring: dispatch-only stream,
            # never queues behind ACT compute) into the f32
            # buffer, cast to bf16, PE-transpose its 4x4 blocks into aT,
            # and feed its k-tiles into G row-block m=0.
            st = state[sl]
            s = sl % SPC
            pool = p_a2 if g < 2 else p_a
            xt = pool.tile([P, 4, C], fp32, tag=f"a_{g}", name=f"a_{sl}_{g}")
            src = x_d[s * N + g * 512 : s * N + (g + 1) * 512, :].rearrange(
                "(u p) c -> p u c", p=P
            )
            a = p_a16.tile([P, 4, C], bf16, tag=f"a16_{g}", name=f"a16_{sl}_{g}")
            if sl == 0 and g == 0:
                # first group: 256KB pieces so the PE starts ~3us earlier
                for u in range(4):
                    nc.sync.dma_start(out=xt[:, u, :], in_=src[:, u, :])
                    nc.scalar.activation(a[:, u, :], xt[:, u, :], AF.Copy)
            else:
                nc.sync.dma_start(out=xt, in_=src)
                nc.scalar.activation(a, xt, AF.Copy)
            st["af"].append(xt)
            st["a16"].append(a)
            # all four G row-blocks accumulate concurrently (4 open PSUM
            # banks), upper triangle only: row m covers d >= 128*m
            for m in range(MT):
                for u in range(4):
                    k = 4 * g + u
                    nc.tensor.matmul(
                        st["psg"][m],
                        a[:, u, m * P : (m + 1) * P],
                        a[:, u, m * P :],
                        start=(k == 0),
                        stop=(k == NT - 1),
                    )
            for j in range(MT):
                pst = pp_t.tile([P, C], fp32, tag="pst", name=f"pst_{sl}_{j}_{g}")
                for u in range(4):
                    nc.tensor.matmul(
                        pst[:, u * P : (u + 1) * P],
                        a[:, u, j * P : (j + 1) * P],
                        ident,
                        start=(u == 0),
                        stop=(u == 3),
                    )
                dstc = st["aT"][j][:, g * 512 : (g + 1) * 512]
                if j >= 2:
                    nc.scalar.activation(dstc, pst, AF.Copy)
                else:
                    nc.vector.tensor_copy(dstc, pst)

        def emit_B(sl):
            # remaining G row-blocks (upper triangle only; G is exactly
            # symmetric), mirror lower blocks via PE transpose, softmax
            # rows, R = gamma*(att - I) in bf16.
            st = state[sl]
            Gs = []
            for m in range(MT):
                psg = st["psg"][m]
                gs = p_gs.tile([P, C], fp32, tag=f"Gs_{m}", name=f"Gs_{sl}_{m}")
                nc.scalar.activation(gs[:, m * P :], psg, AF.Copy)
                for j in range(m):
                    tps = pp_t.tile([P, P], fp32, tag="pst", name=f"tps_{sl}_{m}_{j}")
                    nc.tensor.transpose(tps, Gs[j][:, m * P : (m + 1) * P], identf)
                    nc.scalar.activation(gs[:, j * P : (j + 1) * P], tps, AF.Copy)
                Gs.append(gs)

                nmax = p_st.tile([P, 1], fp32, tag="nmax", name=f"nmax_{sl}_{m}")
                nc.vector.reduce_max(nmax, gs, axis=AX, negate=True)
                E = p_sm.tile([P, C], fp32, tag="E", name=f"E_{sl}_{m}")
                nc.scalar.activation(E, gs, AF.Exp, bias=nmax, scale=1.0)
                ssum = p_st.tile([P, 1], fp32, tag="ssum", name=f"ssum_{sl}_{m}")
                nc.vector.reduce_sum(ssum, E, axis=AX)
                rin = p_st.tile([P, 1], fp32, tag="rin", name=f"rin_{sl}_{m}")
                nc.vector.reciprocal(rin, ssum)
                ag = p_sm.tile([P, C], fp32, tag="ag", name=f"ag_{sl}_{m}")
                nc.vector.tensor_scalar(ag, E, rin, gvec, OP.mult, OP.mult)
                nc.vector.tensor_sub(
                    ag[:, m * P : (m + 1) * P], ag[:, m * P : (m + 1) * P], gI
                )
                r = p_R.tile([P, C], bf16, tag=f"R_{m}", name=f"R_{sl}_{m}")
                nc.scalar.activation(r, ag, AF.Copy)
                st["Rb"].append(r)

        def emit_D_tile(sl, i):
            # tmp = a @ R for n-tile i; z = (1+gamma)*a + tmp fused on
            # DVE, in place over the f32 a buffer; store per 1MB group.
            st = state[sl]
            s = sl % SPC
            po = pp_o.tile([P, C], fp32, tag="po", name=f"po_{sl}_{i}")
            for k in range(MT):
                nc.tensor.matmul(
                    po,
                    st["aT"][k][:, i * P : (i + 1) * P],
                    st["Rb"][k],
                    start=(k == 0),
                    stop=(k == MT - 1),
                )
            gg, vv = divmod(i, 4)
            zsl = st["af"][gg][:, vv, :]
            nc.vector.scalar_tensor_tensor(zsl, zsl, gp1, po, OP.mult, OP.add)
            if vv == 3:
                dst = out_d[
                    s * N + gg * 512 : s * N + (gg + 1) * 512, :
                ].rearrange("(u p) c -> p u c", p=P)
                nc.sync.dma_start(out=dst, in_=st["af"][gg])

        total = SPC * repeat
        for sl in range(total):
            state[sl] = {
                "af": [],
                "a16": [],
                "Rb": [],
                "aT": aT_tiles,
                "psg": [
                    pp_g.tile(
                        [P, C - m * P], fp32, tag="psg", name=f"psg_{sl}_{m}"
                    )
                    for m in range(MT)
                ],
            }
            # Interleave the previous sample's phase D with this sample's
            # load/transpose loop: out2 tiles (4g..4g+3) are the last
            # readers of exactly the aT columns and af group that group g
            # overwrites, so emitting them just ahead keeps every
            # dependency just-in-time.
            for g in range(NG):
                if sl > 0:
                    for t in range(4):
                        emit_D_tile(sl - 1, 4 * g + t)
                emit_A_group(sl, g)
            if sl > 0:
                state.pop(sl - 1)
            emit_B(sl)
        for i in range(NT):
            emit_D_tile(total - 1, i)

    nc.compile()
    return nc


def _get_nc():
    if "nc" not in _CACHE:
        _CACHE["nc"] = _build()
    return _CACHE["nc"]


def _in_maps(x, gamma):
    x = np.asarray(x).astype(np.float32, copy=False)
    g = np.float32(np.asarray(gamma).reshape(-1)[0])
    xs = x.reshape(B, N, C)
    gvec = np.full((P, 1), g, np.float32)
    gp1 = np.full((P, 1), np.float32(1.0) + g, np.float32)
    ident = np.eye(P, dtype=ml_dtypes.bfloat16)
    identf = np.eye(P, dtype=np.float32)
    gi = (g * np.eye(P)).astype(np.float32)
    maps = []
    for r in range(NCORES):
        shard = np.ascontiguousarray(xs[r * SPC : (r + 1) * SPC].reshape(SPC * N, C))
        maps.append(
            {
                "x": shard,
                "gvec": gvec,
                "gp1vec": gp1,
                "ident": ident,
                "identf": identf,
                "gI": gi,
            }
        )
    return maps


def _run(x, gamma, trace=False):
    import os

    if not trace:
        # the NTFF trace hook (antenv.axon_hooks) is absent in this axon
        # build; make sure an inherited BASS_TRACE can't route us there
        os.environ.setdefault("BASS_NEVER_TRACE", "1")
    from concourse import bass_utils

    nc = _get_nc()
    res = bass_utils.run_bass_kernel_spmd(
        nc, _in_maps(x, gamma), core_ids=list(range(NCORES)), trace=trace
    )
    out = np.concatenate(
        [res.results[r]["out"].reshape(SPC, N, C) for r in range(NCORES)], axis=0
    )
    return out.reshape(B, HW_H, HW_W, C).astype(np.float32, copy=False), res


def kernel(x, gamma):
    out, _ = _run(x, gamma, trace=False)
    return out


# revision 31
# speedup vs baseline: 1.4192x; 1.1994x over previous
# Channel-Attention Module (CAM) kernel for Trainium2, 8 NeuronCores.
#
# reference:
#   a   = x.reshape(B, N, C)                 # B=16, N=64*64=4096, C=512
#   G   = einsum('bnc,bnd->bcd', a, a)       # [B, C, C]
#   att = softmax(G, axis=-1)
#   out = gamma * einsum('bnc,bcd->bnd', a, att) + x
#
# Sharding: data-parallel over batch, 2 samples per core.
#
# Numerics: a @ att is rewritten as a @ R + a with R = att - I.  The
# dominant identity contribution is applied as an f32 elementwise op, so
# only a @ R runs through the bf16 tensor engine.  Folding gamma in:
#   out = a @ (gamma*(att - I)) + (1 + gamma) * a
# Both big matmuls (G and a@R) run in bf16 on the PE at full rate; the
# softmax and all elementwise work is f32.
#
# G is symmetric (exactly, even with bf16 inputs), so only the upper
# block-triangle is computed on the PE (d >= 128*m for row-block m); the
# lower blocks are filled in by PE-transposing the mirror blocks while
# assembling full softmax rows in SBUF.
#
# Engine placement: input/output DMAs ride the SP HWDGE ring (a
# dispatch-only stream — queuing them on ACT delays loads behind ACT
# compute), constants go via SWDGE.  Casts/exp and half the transpose
# copybacks on ACT, reductions/adds and the other copybacks on DVE,
# both big matmuls plus the a^T transpose (regular matmul against a
# bf16 identity) on the PE.  All four G row-blocks accumulate in four
# concurrently-open PSUM banks while the load loop streams, so the PE
# never starves waiting for the next 1MB group.

from contextlib import ExitStack

import numpy as np
import ml_dtypes

B = 16
HW_H = 64
HW_W = 64
N = HW_H * HW_W          # 4096 pixels per sample
C = 512                  # channels
NCORES = 8
SPC = B // NCORES        # samples per core
P = 128                  # partitions
MT = C // P              # 4 c-tiles
NT = N // P              # 32 n-tiles per sample
NG = NT // 4             # 8 groups of 4 n-tiles (1MB DMA granularity)

_CACHE = {}


def _build(repeat=1):
    # repeat>1 re-runs the whole shard computation inside one NEFF; used
    # only by the timing harness (work-delta cancels dispatch overhead).
    import concourse.bacc as bacc
    import concourse.tile as tile
    import concourse.mybir as mybir

    fp32 = mybir.dt.float32
    bf16 = mybir.dt.bfloat16
    AX = mybir.AxisListType.X
    OP = mybir.AluOpType
    AF = mybir.ActivationFunctionType

    nc = bacc.Bacc(
        "TRN2",
        target_bir_lowering=False,
        debug=False,
        enable_asserts=False,
        num_devices=NCORES,
    )
    x_d = nc.dram_tensor("x", [SPC * N, C], fp32, kind="ExternalInput").ap()
    gvec_d = nc.dram_tensor("gvec", [P, 1], fp32, kind="ExternalInput").ap()
    gp1_d = nc.dram_tensor("gp1vec", [P, 1], fp32, kind="ExternalInput").ap()
    ident_d = nc.dram_tensor("ident", [P, P], bf16, kind="ExternalInput").ap()
    identf_d = nc.dram_tensor("identf", [P, P], fp32, kind="ExternalInput").ap()
    gi_d = nc.dram_tensor("gI", [P, P], fp32, kind="ExternalInput").ap()
    out_d = nc.dram_tensor("out", [SPC * N, C], fp32, kind="ExternalOutput").ap()

    with tile.TileContext(nc) as tc, ExitStack() as ctx:
        p_const = ctx.enter_context(tc.tile_pool(name="pconst", bufs=1))
        p_a = ctx.enter_context(tc.tile_pool(name="pa", bufs=1))
        p_a2 = ctx.enter_context(tc.tile_pool(name="pa2", bufs=2))
        p_a16 = ctx.enter_context(tc.tile_pool(name="pa16", bufs=1))
        p_aT = ctx.enter_context(tc.tile_pool(name="paT", bufs=1))
        p_gs = ctx.enter_context(tc.tile_pool(name="pgs", bufs=1))
        p_sm = ctx.enter_context(tc.tile_pool(name="psm", bufs=2))
        p_R = ctx.enter_context(tc.tile_pool(name="pR", bufs=1))
        p_st = ctx.enter_context(tc.tile_pool(name="pst", bufs=4))
        pp_g = ctx.enter_context(tc.tile_pool(name="ppg", bufs=4, space="PSUM"))
        pp_t = ctx.enter_context(tc.tile_pool(name="ppt", bufs=2, space="PSUM"))
        pp_o = ctx.enter_context(tc.tile_pool(name="ppo", bufs=2, space="PSUM"))

        gvec = p_const.tile([P, 1], fp32, name="gvec_sb")
        nc.gpsimd.dma_start(out=gvec, in_=gvec_d)
        gp1 = p_const.tile([P, 1], fp32, name="gp1_sb")
        nc.gpsimd.dma_start(out=gp1, in_=gp1_d)
        ident = p_const.tile([P, P], bf16, name="ident_sb")
        nc.gpsimd.dma_start(out=ident, in_=ident_d)
        identf = p_const.tile([P, P], fp32, name="identf_sb")
        nc.gpsimd.dma_start(out=identf, in_=identf_d)
        gI = p_const.tile([P, P], fp32, name="gI_sb")
        nc.gpsimd.dma_start(out=gI, in_=gi_d)

        # aT is allocated once and reused across samples: per-sample
        # re-allocation would add a whole-tile release dependency (next
        # sample's transposes waiting on ALL of the previous out2 reads),
        # while a persistent tile gets byte-range WAR tracking, which the
        # D/A interleave below satisfies just-in-time.
        aT_tiles = [
            p_aT.tile([P, N], bf16, tag=f"aT_{j}", name=f"aT_{j}")
            for j in range(MT)
        ]

        # Per-sample state: af (resident f32 a / z buffers), a16, aT, Rb.
        state = {}

        def emit_A_group(sl, g):
            # load 1MB group g (SP HWDGE ring: dispatch-only stream,
            # never queues behind ACT compute) into the f32
            # buffer, cast to bf16, PE-transpose its 4x4 blocks into aT,
            # and feed its k-tiles into G row-block m=0.
            st = state[sl]
            s = sl % SPC
            pool = p_a2 if g < 2 else p_a
            xt = pool.tile([P, 4, C], fp32, tag=f"a_{g}", name=f"a_{sl}_{g}")
            src = x_d[s * N + g * 512 : s * N + (g + 1) * 512, :].rearrange(
                "(u p) c -> p u c", p=P
            )
            a = p_a16.tile([P, 4, C], bf16, tag=f"a16_{g}", name=f"a16_{sl}_{g}")
            if sl == 0 and g == 0:
                # first group: 256KB pieces so the PE starts ~3us earlier
                for u in range(4):
                    nc.sync.dma_start(out=xt[:, u, :], in_=src[:, u, :])
                    nc.scalar.activation(a[:, u, :], xt[:, u, :], AF.Copy)
            else:
                nc.sync.dma_start(out=xt, in_=src)
                nc.scalar.activation(a, xt, AF.Copy)
            st["af"].append(xt)
            st["a16"].append(a)
            # all four G row-blocks accumulate concurrently (4 open PSUM
            # banks), upper triangle only: row m covers d >= 128*m
            for m in range(MT):
                for u in range(4):
                    k = 4 * g + u
                    nc.tensor.matmul(
                        st["psg"][m],
                        a[:, u, m * P : (m + 1) * P],
                        a[:, u, m * P :],
                        start=(k == 0),
                        stop=(k == NT - 1),
                    )
            for j in range(MT):
                pst = pp_t.tile([P, C], fp32, tag="pst", name=f"pst_{sl}_{j}_{g}")
                for u in range(4):
                    nc.tensor.matmul(
                        pst[:, u * P : (u + 1) * P],
                        a[:, u, j * P : (j + 1) * P],
                        ident,
                        start=(u == 0),
                        stop=(u == 3),
                    )
                dstc = st["aT"][j][:, g * 512 : (g + 1) * 512]
                if j >= 2:
                    nc.scalar.activation(dstc, pst, AF.Copy)
                else:
                    nc.vector.tensor_copy(dstc, pst)

        def emit_B(sl):
            # remaining G row-blocks (upper triangle only; G is exactly
            # symmetric), mirror lower blocks via PE transpose, softmax
            # rows, R = gamma*(att - I) in bf16.
            st = state[sl]
            Gs = []
            for m in range(MT):
                psg = st["psg"][m]
                gs = p_gs.tile([P, C], fp32, tag=f"Gs_{m}", name=f"Gs_{sl}_{m}")
                nc.scalar.activation(gs[:, m * P :], psg, AF.Copy)
                for j in range(m):
                    tps = pp_t.tile([P, P], fp32, tag="pst", name=f"tps_{sl}_{m}_{j}")
                    nc.tensor.transpose(tps, Gs[j][:, m * P : (m + 1) * P], identf)
                    nc.scalar.activation(gs[:, j * P : (j + 1) * P], tps, AF.Copy)
                Gs.append(gs)

                nmax = p_st.tile([P, 1], fp32, tag="nmax", name=f"nmax_{sl}_{m}")
                nc.vector.reduce_max(nmax, gs, axis=AX, negate=True)
                E = p_sm.tile([P, C], fp32, tag="E", name=f"E_{sl}_{m}")
                nc.scalar.activation(E, gs, AF.Exp, bias=nmax, scale=1.0)
                ssum = p_st.tile([P, 1], fp32, tag="ssum", name=f"ssum_{sl}_{m}")
                nc.vector.reduce_sum(ssum, E, axis=AX)
                rin = p_st.tile([P, 1], fp32, tag="rin", name=f"rin_{sl}_{m}")
                nc.vector.reciprocal(rin, ssum)
                ag = p_sm.tile([P, C], fp32, tag="ag", name=f"ag_{sl}_{m}")
                nc.vector.tensor_scalar(ag, E, rin, gvec, OP.mult, OP.mult)
                nc.vector.tensor_sub(
                    ag[:, m * P : (m + 1) * P], ag[:, m * P : (m + 1) * P], gI
                )
                r = p_R.tile([P, C], bf16, tag=f"R_{m}", name=f"R_{sl}_{m}")
                nc.scalar.activation(r, ag, AF.Copy)
                st["Rb"].append(r)

        def emit_D_tile(sl, i):
            # tmp = a @ R for n-tile i; z = (1+gamma)*a + tmp fused on
            # DVE, in place over the f32 a buffer; store per 1MB group.
            st = state[sl]
            s = sl % SPC
            po = pp_o.tile([P, C], fp32, tag="po", name=f"po_{sl}_{i}")
            for k in range(MT):
                nc.tensor.matmul(
                    po,
                    st["aT"][k][:, i * P : (i + 1) * P],
                    st["Rb"][k],
                    start=(k == 0),
                    stop=(k == MT - 1),
                )
            gg, vv = divmod(i, 4)
            zsl = st["af"][gg][:, vv, :]
            nc.vector.scalar_tensor_tensor(zsl, zsl, gp1, po, OP.mult, OP.add)
            if vv == 3:
                dst = out_d[
                    s * N + gg * 512 : s * N + (gg + 1) * 512, :
                ].rearrange("(u p) c -> p u c", p=P)
                nc.sync.dma_start(out=dst, in_=st["af"][gg])

        total = SPC * repeat
        for sl in range(total):
            state[sl] = {
                "af": [],
                "a16": [],
                "Rb": [],
                "aT": aT_tiles,
                "psg": [
                    pp_g.tile(
                        [P, C - m * P], fp32, tag="psg", name=f"psg_{sl}_{m}"
                    )
                    for m in range(MT)
                ],
            }
            # Interleave the previous sample's phase D with this sample's
            # load/transpose loop: out2 tiles (4g..4g+3) are the last
            # readers of exactly the aT columns and af group that group g
            # overwrites, so emitting them just ahead keeps every
            # dependency just-in-time.
            for g in range(NG):
                if sl > 0:
                    for t in range(4):
                        emit_D_tile(sl - 1, 4 * g + t)
                emit_A_group(sl, g)
            if sl > 0:
                state.pop(sl - 1)
            emit_B(sl)
        for i in range(NT):
            emit_D_tile(total - 1, i)

    nc.compile()
    return nc


def _get_nc():
    if "nc" not in _CACHE:
        _CACHE["nc"] = _build()
    return _CACHE["nc"]


def _in_maps(x, gamma):
    x = np.asarray(x).astype(np.float32, copy=False)
    g = np.float32(np.asarray(gamma).reshape(-1)[0])
    xs = x.reshape(B, N, C)
    gvec = np.full((P, 1), g, np.float32)
    gp1 = np.full((P, 1), np.float32(1.0) + g, np.float32)
    ident = np.eye(P, dtype=ml_dtypes.bfloat16)
    identf = np.eye(P, dtype=np.float32)
    gi = (g * np.eye(P)).astype(np.float32)
    maps = []
    for r in range(NCORES):
        shard = np.ascontiguousarray(xs[r * SPC : (r + 1) * SPC].reshape(SPC * N, C))
        maps.append(
            {
                "x": shard,
                "gvec": gvec,
                "gp1vec": gp1,
                "ident": ident,
                "identf": identf,
                "gI": gi,
            }
        )
    return maps


def _run(x, gamma, trace=False):
    import os

    if not trace:
        # the NTFF trace hook (antenv.axon_hooks) is absent in this axon
        # build; make sure an inherited BASS_TRACE can't route us there
        os.environ.setdefault("BASS_NEVER_TRACE", "1")
    from concourse import bass_utils

    nc = _get_nc()
    res = bass_utils.run_bass_kernel_spmd(
        nc, _in_maps(x, gamma), core_ids=list(range(NCORES)), trace=trace
    )
    out = np.concatenate(
        [res.results[r]["out"].reshape(SPC, N, C) for r in range(NCORES)], axis=0
    )
    return out.reshape(B, HW_H, HW_W, C).astype(np.float32, copy=False), res


def kernel(x, gamma):
    out, _ = _run(x, gamma, trace=False)
    return out
